# revision 16
# baseline (speedup 1.0000x reference)
"""Trainium2 Bass kernel for nn_CrossAttentionFusion (2-layer cross-attention
transformer block).

Sharding: 8 cores = 4 batches x 2-way split of the 512 query rows.  Each core
owns one batch's context (KV duplicated inside the pair; no collectives) and
256 query rows.

Layout strategy (per core):
  - residual stream q kept natural fp32 [tok(2x128), C]
  - context/query passed in host-pre-transposed + bf16
  - cT = Wcp^T @ ctxT kept [C-part, NC-free] bf16 (both layers)
  - scores computed transposed [NC-part, tok-free] so mask+scale+exp is one
    fused ScalarE activation with a per-partition bias, no max-subtraction
  - softmax denominator = extra ones-column in the AV matmul rhs, normalized
    per-partition after accumulation
  - FFN hidden transposed [4096-part, tok-free]; zero transposes in the FFN
  - LN gamma/beta folded into the following matmul weights on the host
"""

import types

import numpy as np
import ml_dtypes
import orjson

import concourse.bass as bass
import concourse.mybir as mybir
import concourse.tile as tile
from concourse.bass_utils import run_bass_kernel_spmd
from concourse.masks import make_identity

FP32 = mybir.dt.float32
BF16 = mybir.dt.bfloat16
FP8 = mybir.dt.float8e4
DR = mybir.MatmulPerfMode.DoubleRow
AF = mybir.ActivationFunctionType
ALU = mybir.AluOpType

# fp8 scale folding (all powers of 2; every descale folds into host-side
# weight scaling, the qh projection, or the softmax denominator column):
#   cT  = 16*c      (ctxT natural fp8, wcp*16)
#   kTg = 512*k     (wkk*32; qh side folded /512 on host)
#   vxg = 64*v      (wkv*4; ones column = 64 so AV normalization cancels)
SC_CT = 16.0
SC_WK = 32.0
SC_WV = 4.0
VONES = SC_CT * SC_WV  # 64: scale of vxg including denominator column

B, NQ, NCTX = 4, 512, 2048
DQ, DC, C = 1024, 768, 1024
H, HD, L = 16, 64, 2
FF = 4 * C
EPS = 1e-5
P = 128
TOK = NQ // 2          # 256 query rows per core
NT = NCTX // P         # 16 context tiles
CT = C // P            # 8
DCT = DC // P          # 6
N_CORES = 8
NEG = -30000.0         # mask bias; exp underflows to exactly 0


def _split_waits_json_bytes(nc):
    """This walrus build supports a single sync-wait per instruction; move
    extra waits emitted by Tile onto preceding NoOps on the same engine."""
    raw = bass.Bass.to_json_bytes(nc)
    bir = orjson.loads(raw)
    for fn in bir.get("functions", []):
        for blk in fn.get("blocks", []):
            out = []
            for ins in blk.get("instructions", []):
                si = ins.get("sync_info")
                if si:
                    waits = si.get("on_wait") or []
                    if len(waits) > 1:
                        for j, w in enumerate(waits[:-1]):
                            out.append({
                                "name": ins["name"] + f"_ws{j}",
                                "opcode": "NoOp",
                                "engine": ins["engine"],
                                "ins": [], "outs": [],
                                "sync_info": {"on_update": [], "on_wait": [w]},
                                "debug": ins.get("debug", 0),
                            })
                        si["on_wait"] = waits[-1:]
                out.append(ins)
            blk["instructions"] = out
    return orjson.dumps(bir)


def build_program(loop_n=None, sim_safe=False, stub=False):
    import contextlib
    gelu_fn = AF.Identity if sim_safe else AF.Gelu
    nc = bass.Bass()

    def din(name, shape, dt=BF16):
        return nc.dram_tensor(name, shape, dt, kind="ExternalInput")

    qT = din("qT", [DQ, TOK])
    ctxT = din("ctxT", [DC, NCTX], FP8)
    maskb = din("maskb", [NCTX], FP32)
    wqp = din("wqp", [DQ, C]); bqp = din("bqp", [C], FP32)
    wcp = din("wcp", [DC, C], FP8); bcp = din("bcp", [C], FP32)
    wq = din("wq", [L, C, C]); bq = din("bq", [L, C], FP32)
    wkk = din("wkk", [L, C, C], FP8); bkk = din("bkk", [L, C], FP32)
    wkv = din("wkv", [L, C, C], FP8); bkv = din("bkv", [L, C], FP32)
    wo = din("wo", [L, C, C]); bo = din("bo", [L, C], FP32)
    w1 = din("w1", [L, C, FF]); bf1 = din("bf1", [L, FF], FP32)
    w2 = din("w2", [L, FF, C]); bf2 = din("bf2", [L, C], FP32)
    gf = din("gf", [C], FP32); betaf = din("betaf", [C], FP32)
    out = nc.dram_tensor("out", [TOK, C], FP32, kind="ExternalOutput")

    with tile.TileContext(nc) as tc:
        with (
            tc.tile_pool(name="singles", bufs=1) as singles,
            tc.tile_pool(name="persist", bufs=1) as persist,
            tc.tile_pool(name="wpool", bufs=2) as wpool,
            tc.tile_pool(name="xstage", bufs=2) as xstage,
            tc.tile_pool(name="qhp", bufs=1) as qhp,
            tc.tile_pool(name="etp", bufs=2) as etp,
            tc.tile_pool(name="aop", bufs=1) as aop,
            tc.tile_pool(name="g1p", bufs=2) as g1p,
            tc.tile_pool(name="biasp", bufs=2) as biasp,
            tc.tile_pool(name="small", bufs=4) as small,
            tc.tile_pool(name="lnp", bufs=2) as lnp,
            tc.tile_pool(name="psmm", bufs=2, space="PSUM") as psmm,
            tc.tile_pool(name="pssc", bufs=2, space="PSUM") as pssc,
            tc.tile_pool(name="psav", bufs=2, space="PSUM") as psav,
        ):
          with (tc.For_i(0, loop_n, 1) if loop_n else contextlib.nullcontext()):
            ident = singles.tile([P, P], BF16, tag="ident")
            make_identity(nc, ident)
            eps_t = singles.tile([P, 1], FP32, tag="eps")
            nc.vector.memset(eps_t, EPS)
            maskb_sb = singles.tile([P, NT], FP32, tag="maskb")
            nc.sync.dma_start(maskb_sb, maskb.rearrange("(o p) -> p o", p=P))
            bcp_sb = singles.tile([P, CT], FP32, tag="bcp")
            nc.sync.dma_start(bcp_sb, bcp.rearrange("(o p) -> p o", p=P))

            qres = persist.tile([P, 2, C], FP32, tag="qres")
            cT = persist.tile([P, CT, NCTX], FP8, tag="cT")

            def ln_normalize(dst, src):
                """dst (bf16 [P, C]) = (src - mean) * rsqrt(var + eps)"""
                st = small.tile([P, 2, 6], FP32, tag="lnstats")
                nc.vector.bn_stats(st[:, 0], src[:, 0:512])
                nc.vector.bn_stats(st[:, 1], src[:, 512:1024])
                mv = small.tile([P, 2], FP32, tag="lnmv")
                nc.vector.bn_aggr(mv, st)
                std = small.tile([P, 1], FP32, tag="lnstd")
                nc.scalar.activation(std, mv[:, 1:2], AF.Sqrt, bias=eps_t[:, 0:1])
                rstd = small.tile([P, 1], FP32, tag="lnrstd")
                nc.vector.reciprocal(rstd, std)
                nc.vector.tensor_scalar(
                    dst, src, scalar1=mv[:, 0:1], scalar2=rstd,
                    op0=ALU.subtract, op1=ALU.mult)

            def transpose_rows(dst, src, tt):
                """src bf16 [P(tok), C] -> dst bf16 [P, CT, TOK] cols tt*128.."""
                for ct in range(CT):
                    ps = psmm.tile([P, P], BF16, tag="pstr")
                    nc.tensor.transpose(ps, src[:, ct * P:(ct + 1) * P], ident)
                    nc.vector.tensor_copy(dst[:, ct, tt * P:(tt + 1) * P], ps)

            # ---------------- preamble: q = query @ Wqp + bqp -------------
            with tc.tile_pool(name="pre", bufs=1) as pre:
                qT_sb = pre.tile([P, DQ // P, TOK], BF16, tag="preqT")
                nc.sync.dma_start(qT_sb, qT.rearrange("(o p) t -> p o t", p=P))
                wqp_sb = wpool.tile([P, DQ // P, C], BF16, tag="w")
                nc.sync.dma_start(wqp_sb, wqp.rearrange("(o p) c -> p o c", p=P))
                bqp_bc = biasp.tile([P, C], FP32, tag="biasf")
                nc.gpsimd.dma_start(bqp_bc, bqp[None, :].to_broadcast([P, C]))
                for tt in range(2):
                    for nn in range(2):
                        ps = psmm.tile([P, 512], FP32, tag="psmm")
                        for kc in range(DQ // P):
                            nc.tensor.matmul(
                                ps, qT_sb[:, kc, tt * P:(tt + 1) * P],
                                wqp_sb[:, kc, nn * 512:(nn + 1) * 512],
                                start=(kc == 0), stop=(kc == DQ // P - 1))
                        nc.vector.tensor_add(
                            qres[:, tt, nn * 512:(nn + 1) * 512], ps,
                            bqp_bc[:, nn * 512:(nn + 1) * 512])

                # ---------- cT = 16*(context @ Wcp + bcp)^T  (fp8 DR) ------
                ctxT_sb = pre.tile([P, DCT, NCTX], FP8, tag="prectxT")
                nc.sync.dma_start(ctxT_sb, ctxT.rearrange("(o p) n -> p o n", p=P))
                wcp_sb = wpool.tile([P, DCT, C], FP8, tag="w8c")
                nc.sync.dma_start(wcp_sb, wcp.rearrange("(o p) c -> p o c", p=P))
                for ct in range(CT):
                    for nn in range(4):
                        ps = psmm.tile([P, 512], FP32, tag="psmm")
                        for kc in range(DCT // 2):
                            nc.tensor.matmul(
                                ps, wcp_sb[:, 2 * kc:2 * kc + 2,
                                           ct * P:(ct + 1) * P],
                                ctxT_sb[:, 2 * kc:2 * kc + 2,
                                        nn * 512:(nn + 1) * 512],
                                start=(kc == 0), stop=(kc == DCT // 2 - 1),
                                perf_mode=DR)
                        nc.scalar.activation(
                            cT[:, ct, nn * 512:(nn + 1) * 512], ps, AF.Identity,
                            bias=bcp_sb[:, ct:ct + 1])

            kTg = persist.tile([P, 4, NCTX], BF16, tag="kTg")
            vxg = persist.tile([P, NT, H // 2, HD + 1], FP8, tag="vxg")

            for l in range(0 if stub else L):
                # ---------- LN1 -> qnT ------------------------------------
                qnT = xstage.tile([P, CT, TOK], BF16, tag="xT")
                for tt in range(2):
                    qn = lnp.tile([P, C], BF16, tag="ln")
                    ln_normalize(qn, qres[:, tt, :])
                    transpose_rows(qnT, qn, tt)

                # ---------- qhT = (Wq'^T @ qnT) + bq'  (scale folded) -----
                wq_sb = wpool.tile([P, CT, C], BF16, tag="w")
                nc.sync.dma_start(wq_sb, wq[l].rearrange("(o p) c -> p o c", p=P))
                bq_sb = small.tile([P, CT], FP32, tag="pb")
                nc.sync.dma_start(bq_sb, bq[l].rearrange("(o p) -> p o", p=P))
                qhT = qhp.tile([P, CT, TOK], BF16, tag="qhT")
                for ct in range(CT):
                    ps = psmm.tile([P, TOK], FP32, tag="psmm")
                    for cc in range(CT):
                        nc.tensor.matmul(
                            ps, wq_sb[:, cc, ct * P:(ct + 1) * P], qnT[:, cc, :],
                            start=(cc == 0), stop=(cc == CT - 1))
                    nc.scalar.activation(qhT[:, ct, :], ps, AF.Identity,
                                         bias=bq_sb[:, ct:ct + 1])

                wo_sb = wpool.tile([P, CT, C], BF16, tag="w")
                nc.sync.dma_start(wo_sb, wo[l].rearrange("(o p) c -> p o c", p=P))
                bo_bc = biasp.tile([P, C], FP32, tag="biasf")
                nc.gpsimd.dma_start(bo_bc, bo[l][None, :].to_broadcast([P, C]))
                aout = aop.tile([P, 2, C], BF16, tag="aout")

                for g in range(2):  # head groups of 8 (split kv to save SBUF)
                    # ---- kTg: k-dims g*512..g*512+511 over all NC --------
                    wk_sb = wpool.tile([P, CT, 512], FP8, tag="w8")
                    nc.sync.dma_start(
                        wk_sb, wkk[l][:, g * 512:(g + 1) * 512]
                        .rearrange("(o p) c -> p o c", p=P))
                    bkk_sb = small.tile([P, CT], FP32, tag="pb2")
                    nc.sync.dma_start(bkk_sb, bkk[l].rearrange("(o p) -> p o", p=P))
                    for kt in range(4):
                        for nn in range(4):
                            ps = psmm.tile([P, 512], FP32, tag="psmm")
                            for cc in range(CT // 2):
                                nc.tensor.matmul(
                                    ps, wk_sb[:, 2 * cc:2 * cc + 2,
                                              kt * P:(kt + 1) * P],
                                    cT[:, 2 * cc:2 * cc + 2,
                                       nn * 512:(nn + 1) * 512],
                                    start=(cc == 0), stop=(cc == CT // 2 - 1),
                                    perf_mode=DR)
                            nc.vector.tensor_scalar_add(
                                kTg[:, kt, nn * 512:(nn + 1) * 512], ps,
                                bkk_sb[:, g * 4 + kt:g * 4 + kt + 1])

                    # ---- v columns g*512.. -> vxg [P, NT, 8, HD+1] -------
                    wv_sb = wpool.tile([P, CT, 512], FP8, tag="w8")
                    nc.sync.dma_start(
                        wv_sb, wkv[l][:, g * 512:(g + 1) * 512]
                        .rearrange("(o p) c -> p o c", p=P))
                    bkv_bc = biasp.tile([P, C], FP32, tag="biasf")
                    nc.gpsimd.dma_start(bkv_bc, bkv[l][None, :].to_broadcast([P, C]))
                    nc.vector.memset(vxg[:, :, :, HD:HD + 1], VONES)
                    for nt in range(NT):
                        ps = psmm.tile([P, 512], FP32, tag="psmm")
                        for cc in range(CT // 2):
                            nc.tensor.matmul(
                                ps, cT[:, 2 * cc:2 * cc + 2,
                                       nt * P:(nt + 1) * P],
                                wv_sb[:, 2 * cc:2 * cc + 2, :],
                                start=(cc == 0), stop=(cc == CT // 2 - 1),
                                perf_mode=DR)
                        nc.vector.tensor_add(
                            vxg[:, nt, :, :HD],
                            ps.rearrange("p (h d) -> p h d", d=HD),
                            bkv_bc[:, g * 512:(g + 1) * 512]
                            .rearrange("p (h d) -> p h d", d=HD))
                    # zero masked context rows (v AND denominator column):
                    # exact equivalent of -inf score masking, frees the exp
                    # bias port so exp can batch 4 tiles per instruction
                    for nt in range(NT):
                        nc.vector.tensor_scalar_mul(
                            vxg[:, nt], vxg[:, nt], maskb_sb[:, nt:nt + 1])

                    # ---- attention: software-pipelined over the 8 heads --
                    def emit_scores(j):
                        jt = j // 2
                        off = (j % 2) * HD
                        eT = etp.tile([P, NT, TOK], FP8, tag="eT")
                        for q2 in range(NT // 2):
                            ps2 = pssc.tile([P, 2, TOK], FP32, tag="pssc")
                            for k2 in range(2):
                                nt = 2 * q2 + k2
                                nc.tensor.matmul(
                                    ps2[:, k2, :],
                                    kTg[off:off + HD, jt, nt * P:(nt + 1) * P],
                                    qhT[off:off + HD, g * 4 + jt, :],
                                    start=True, stop=True)
                            nc.scalar.activation(
                                eT[:, 2 * q2:2 * q2 + 2, :], ps2, AF.Exp)
                        return eT

                    def emit_av(j, eT):
                        h = g * 8 + j
                        for tt in range(2):
                            av = psav.tile([P, HD + 1], FP32, tag="psav")
                            for nt2 in range(NT // 2):
                                nc.tensor.matmul(
                                    av, eT[:, 2 * nt2:2 * nt2 + 2,
                                           tt * P:(tt + 1) * P],
                                    vxg[:, 2 * nt2:2 * nt2 + 2, j, :],
                                    start=(nt2 == 0), stop=(nt2 == NT // 2 - 1),
                                    perf_mode=DR)
                            rden = small.tile([P, 1], FP32, tag="rden")
                            nc.vector.reciprocal(rden, av[:, HD:HD + 1])
                            nc.vector.tensor_scalar_mul(
                                aout[:, tt, h * HD:(h + 1) * HD],
                                av[:, :HD], rden)

                    prev = None
                    for j in range(H // 2):
                        eT_j = emit_scores(j)
                        if prev is not None:
                            emit_av(*prev)
                        prev = (j, eT_j)
                    emit_av(*prev)

                # ---------- attn out -> aoT; q += aoT^T Wo + bo -----------
                aoT = xstage.tile([P, CT, TOK], BF16, tag="aoT")
                for tt in range(2):
                    transpose_rows(aoT, aout[:, tt, :], tt)
                for tt in range(2):
                    nc.vector.tensor_add(qres[:, tt, :], qres[:, tt, :], bo_bc)
                    for nn in range(2):
                        ps = psmm.tile([P, 512], FP32, tag="psmm")
                        for cc in range(CT):
                            nc.tensor.matmul(
                                ps, aoT[:, cc, tt * P:(tt + 1) * P],
                                wo_sb[:, cc, nn * 512:(nn + 1) * 512],
                                start=(cc == 0), stop=(cc == CT - 1))
                        nc.vector.tensor_add(
                            qres[:, tt, nn * 512:(nn + 1) * 512],
                            qres[:, tt, nn * 512:(nn + 1) * 512], ps)

                # ---------- LN2 -> hT; FFN (hidden transposed) ------------
                hT = xstage.tile([P, CT, TOK], BF16, tag="xT")
                for tt in range(2):
                    hn = lnp.tile([P, C], BF16, tag="ln")
                    ln_normalize(hn, qres[:, tt, :])
                    transpose_rows(hT, hn, tt)

                bf1_sb = small.tile([P, FF // P], FP32, tag="pb3")
                nc.sync.dma_start(bf1_sb, bf1[l].rearrange("(o p) -> p o", p=P))
                bf2_bc = biasp.tile([P, C], FP32, tag="biasf")
                nc.gpsimd.dma_start(bf2_bc, bf2[l][None, :].to_broadcast([P, C]))
                for tt in range(2):
                    nc.vector.tensor_add(qres[:, tt, :], qres[:, tt, :], bf2_bc)

                for qd in range(4):  # FF quarters of 1024
                    w1_sb = wpool.tile([P, CT, C], BF16, tag="w")
                    nc.sync.dma_start(
                        w1_sb, w1[l][:, qd * C:(qd + 1) * C]
                        .rearrange("(o p) f -> p o f", p=P))
                    g1T = g1p.tile([P, 8, TOK], BF16, tag="g1T")
                    for j in range(8):
                        ps = psmm.tile([P, TOK], FP32, tag="psmm")
                        for cc in range(CT):
                            nc.tensor.matmul(
                                ps, w1_sb[:, cc, j * P:(j + 1) * P], hT[:, cc, :],
                                start=(cc == 0), stop=(cc == CT - 1))
                        nc.scalar.activation(
                            g1T[:, j, :], ps, gelu_fn,
                            bias=bf1_sb[:, qd * 8 + j:qd * 8 + j + 1])
                    w2_sb = wpool.tile([P, CT, C], BF16, tag="w")
                    nc.sync.dma_start(
                        w2_sb, w2[l][qd * C:(qd + 1) * C, :]
                        .rearrange("(o p) c -> p o c", p=P))
                    for tt in range(2):
                        for nn in range(2):
                            ps = psmm.tile([P, 512], FP32, tag="psmm")
                            for hc in range(8):
                                nc.tensor.matmul(
                                    ps, g1T[:, hc, tt * P:(tt + 1) * P],
                                    w2_sb[:, hc, nn * 512:(nn + 1) * 512],
                                    start=(hc == 0), stop=(hc == 7))
                            nc.vector.tensor_add(
                                qres[:, tt, nn * 512:(nn + 1) * 512],
                                qres[:, tt, nn * 512:(nn + 1) * 512], ps)

            # ---------------- final LN * gf + betaf -----------------------
            gf_bc = biasp.tile([P, C], FP32, tag="biasf")
            nc.gpsimd.dma_start(gf_bc, gf[None, :].to_broadcast([P, C]))
            betaf_bc = biasp.tile([P, C], FP32, tag="biasf")
            nc.gpsimd.dma_start(betaf_bc, betaf[None, :].to_broadcast([P, C]))
            for tt in range(2):
                on = lnp.tile([P, C], FP32, tag="lnout")
                # fp32 LN output (no bf16 rounding on the final result)
                st = small.tile([P, 2, 6], FP32, tag="lnstats")
                nc.vector.bn_stats(st[:, 0], qres[:, tt, 0:512])
                nc.vector.bn_stats(st[:, 1], qres[:, tt, 512:1024])
                mv = small.tile([P, 2], FP32, tag="lnmv")
                nc.vector.bn_aggr(mv, st)
                std = small.tile([P, 1], FP32, tag="lnstd")
                nc.scalar.activation(std, mv[:, 1:2], AF.Sqrt, bias=eps_t[:, 0:1])
                rstd = small.tile([P, 1], FP32, tag="lnrstd")
                nc.vector.reciprocal(rstd, std)
                nc.vector.tensor_scalar(
                    on, qres[:, tt, :], scalar1=mv[:, 0:1], scalar2=rstd,
                    op0=ALU.subtract, op1=ALU.mult)
                nc.vector.tensor_mul(on, on, gf_bc)
                nc.vector.tensor_add(on, on, betaf_bc)
                nc.sync.dma_start(out[tt * P:(tt + 1) * P, :], on)

    nc.to_json_bytes = types.MethodType(_split_waits_json_bytes, nc)
    return nc


_PROGRAM = None


def _get_program():
    global _PROGRAM
    if _PROGRAM is None:
        _PROGRAM = build_program()
    return _PROGRAM


def _bf(x):
    return np.ascontiguousarray(np.asarray(x, np.float32)).astype(ml_dtypes.bfloat16)


def _f8(x):
    return np.ascontiguousarray(np.asarray(x, np.float32)).astype(
        ml_dtypes.float8_e4m3)


def _f32(x):
    return np.ascontiguousarray(np.asarray(x, np.float32))


def prepare_inputs(query, context, context_mask, Wqp, bqp, Wcp, bcp, Wq, bq,
                   Wkv, bkv, Wo, bo, g1, beta1, W1, bf1, W2, bf2, g2, beta2,
                   gf, betaf):
    """Host-side prep: fold LN affine params into following matmuls, fold the
    attention scale into Wq, split Wkv, transpose activations, cast to bf16."""
    scale = HD ** -0.5
    q32 = np.asarray(query, np.float32)
    c32 = np.asarray(context, np.float32)
    mask = np.asarray(context_mask)
    Wq = np.asarray(Wq, np.float32); bq = np.asarray(bq, np.float32)
    g1 = np.asarray(g1, np.float32); beta1 = np.asarray(beta1, np.float32)
    W1 = np.asarray(W1, np.float32); bf1 = np.asarray(bf1, np.float32)
    g2 = np.asarray(g2, np.float32); beta2 = np.asarray(beta2, np.float32)
    Wkv = np.asarray(Wkv, np.float32); bkv_full = np.asarray(bkv, np.float32)

    # qh side absorbs the 1/(SC_CT*SC_WK) descale of kTg = 512*k
    wq_f = np.einsum("lc,lcd->lcd", g1, Wq) * (scale / (SC_CT * SC_WK))
    bq_f = (np.einsum("lc,lcd->ld", beta1, Wq) + bq) * (scale / (SC_CT * SC_WK))
    w1_f = np.einsum("lc,lcf->lcf", g2, W1)
    bf1_f = np.einsum("lc,lcf->lf", beta2, W1) + bf1

    shared = {
        "wqp": _bf(Wqp), "bqp": _f32(bqp),
        "wcp": _f8(SC_CT * Wcp), "bcp": _f32(SC_CT * np.asarray(bcp, np.float32)),
        "wq": _bf(wq_f), "bq": _f32(bq_f),
        "wkk": _f8(SC_WK * Wkv[:, :, :C]),
        "bkk": _f32(SC_CT * SC_WK * bkv_full[:, :C]),
        "wkv": _f8(SC_WV * Wkv[:, :, C:]),
        "bkv": _f32(SC_CT * SC_WV * bkv_full[:, C:]),
        "wo": _bf(Wo), "bo": _f32(bo),
        "w1": _bf(w1_f), "bf1": _f32(bf1_f),
        "w2": _bf(W2), "bf2": _f32(bf2),
        "gf": _f32(gf), "betaf": _f32(betaf),
    }
    in_maps = []
    for core in range(N_CORES):
        b, half = core // 2, core % 2
        m = dict(shared)
        m["qT"] = _bf(q32[b, half * TOK:(half + 1) * TOK, :].T)
        m["ctxT"] = _f8(c32[b].T)
        m["maskb"] = _f32(mask[b] != 0)  # 1.0 keep / 0.0 drop, applied to vxg
        in_maps.append(m)
    return in_maps


def kernel(**inputs):
    nc = _get_program()
    in_maps = prepare_inputs(**inputs)
    res = run_bass_kernel_spmd(nc, in_maps, list(range(N_CORES)))
    out = np.empty((B, NQ, C), np.float32)
    for core in range(N_CORES):
        b, half = core // 2, core % 2
        out[b, half * TOK:(half + 1) * TOK, :] = res.results[core]["out"]
    return out

